# revision 23
# baseline (speedup 1.0000x reference)
"""GateRecurrent2dnoind (horizontal, forward) Trainium2 kernel, v6.

Semantics (matching the reference):
  G1u, G2u = bilinear 2x upsample (half-pixel) of G1, G2 to (256, 256)
  g1x = G1u * X
  o = g1x; repeat 128x: o = g1x + G2u * shift_right_w(o)   (left edge replicated)

The 128 Jacobi passes collapse into ONE sequential scan along W with an exact
depth-128 window emulation:
  s[x] = d[x] + g2u[x]*s[x-1],  d = g1u*X  with
  d[0] scaled by s0c = 1 + a0*sum_{m=0}^{127} a0^m   (a0 = g2u[.,0]) and
  d[x] -= q[x] for x=1..K, q[x] = (b0*qc)*prod_{i=1..x}(g2u[i]*rec),
  qc = mask(a0>=T)*a0^129, rec = 1/max(a0,T)  (window-excess correction).

Design highlights (measured on HW):
  - fp16 end-to-end for the big tensors (rel err 1.5e-3 vs the 2e-2 gate).
  - Both upsample directions run on the TensorEngine: per 2KB PSUM bank,
    matmul#1 (weights 3U, moving AP with a stride-0 repeat dim) writes the
    center tap to both W-parity slots, matmul#2 (weights U, a (j:+1, rep:+2)
    window over host-padded 130-col G tiles) accumulates the side taps.
  - The main scan reads g2u straight from PSUM (fp32 data0 = 2 cyc/elem;
    16-bit data0 would be 4 cyc/elem), data1 = d fp16, out fp16.
  - The ENTIRE correction chain (b0, spacers, qd, and all 32 correction
    scans) is precomputed per parity from G1/G2/X column slices, so the
    steady-state loop is only: 8 matmuls (PE), g1u cast (ACT), memset +
    2-piece d-multiply + main scan (DVE), col0 copy + subtract (GpSimd).

Sharding: batch b -> core b (8 batches, 8 cores). Per core: [64, 256, 256].
"""

import numpy as np

import concourse.bacc as bacc
import concourse.mybir as mybir
import concourse.tile as tile
from concourse.ap import AP
from concourse.bass_utils import run_bass_kernel_spmd

f32 = mybir.dt.float32
f16 = mybir.dt.float16
Alu = mybir.AluOpType

NCORES = 8
C = 64          # channels per core
H = 256
W = 256
HG = 128        # G input h/w
WPAD = HG + 2   # G w + replicate pads
B = 4           # channels per block
NBLK = C // B
K = 16          # correction columns
KP = K + 1      # correction scan width per channel
THRESH = 0.75   # a0 mask/clamp for the correction chain
NG2C = 11       # padded G2 columns needed for g2u[0..17] (pad + cols 0..9)


def _upsample_mats():
    """[k=in_row, m=out_row] H-upsample matrices, scaled by 0.25.

    even rows: out[m] = 0.25*in[m-1] + 0.75*in[m]   (m=0 clamps to in[0])
    odd rows:  out[m] = 0.75*in[m] + 0.25*in[m+1]   (m=127 clamps to in[127])
    """
    ue = np.zeros((HG, HG), np.float32)
    uo = np.zeros((HG, HG), np.float32)
    for m in range(HG):
        ue[m, m] += 0.25 * 0.75
        ue[max(m - 1, 0), m] += 0.25 * 0.25
        uo[m, m] += 0.25 * 0.75
        uo[min(m + 1, HG - 1), m] += 0.25 * 0.25
    return ue, uo


def _rep_ap(anchor, dims):
    """Raw AP sharing anchor's tensor/offset/partition dim, custom free dims."""
    return AP(anchor.tensor, anchor.offset, [list(anchor.ap[0])] + dims)


def _precompute(nc, ps2p, constp, weights, Auxd):
    """Per-parity boundary coefficients + the full correction tables.

    Aux layout (host-packed, [128, 2+NG2C+2 per channel] fp16 c-major):
      [c,0] = G1[c,:,0]; [c,1] = G2[c,:,0]; [c,2:2+NG2C] = padded G2 head;
      [c,-2] = X[c,0::2,0]; [c,-1] = X[c,1::2,0].
    Returns {par: (d0_all [128,C] f16, qo_all [128,C*KP] f16)}.
    """
    FA = 2 + NG2C + 2
    aux = constp.tile([HG, C * FA], f16, tag="aux")
    nc.sync.dma_start(aux[:], Auxd[:])
    auxr = aux[:].rearrange("p (c f) -> p c f", c=C)
    g1c0 = auxr[:, :, 0]
    g2c0 = auxr[:, :, 1]
    ghr = auxr[:, :, 2:2 + NG2C]

    out = {}
    for par in ("e", "o"):
        u1 = weights[par + "1"]
        xc0 = auxr[:, :, FA - 2 if par == "e" else FA - 1]

        # a0 = 4*(U @ g2c0); a1_0 = 4*(U @ g1c0)  (= g1u[.,0])
        ps = ps2p.tile([HG, C], f32, tag="ps2")
        nc.tensor.matmul(ps[:], u1[:], g2c0, start=True, stop=True)
        a0 = constp.tile([HG, C], f32, tag=f"a0{par}")
        nc.vector.tensor_scalar_mul(a0[:], ps[:], 4.0)
        ps1b = ps2p.tile([HG, C], f32, tag="ps2")
        nc.tensor.matmul(ps1b[:], u1[:], g1c0, start=True, stop=True)
        b0 = constp.tile([HG, C], f32, tag=f"b0{par}")
        nc.vector.tensor_scalar_mul(b0[:], ps1b[:], 4.0)
        xc0f = constp.tile([HG, C], f32, tag=f"xc0f{par}")
        nc.vector.tensor_copy(xc0f[:], xc0)
        nc.vector.tensor_tensor(b0[:], b0[:], xc0f[:], Alu.mult)

        # geo = sum_{m=0}^{127} a0^m = prod_k (1 + a0^(2^k)), k=0..6
        acc = constp.tile([HG, C], f32, tag=f"acc{par}")
        p = constp.tile([HG, C], f32, tag=f"p{par}")
        t = constp.tile([HG, C], f32, tag=f"t{par}")
        nc.vector.tensor_scalar_add(acc[:], a0[:], 1.0)
        nc.vector.tensor_tensor(p[:], a0[:], a0[:], Alu.mult)
        for _ in range(5):
            nc.vector.tensor_scalar_add(t[:], p[:], 1.0)
            nc.vector.tensor_tensor(acc[:], acc[:], t[:], Alu.mult)
            nc.vector.tensor_tensor(p[:], p[:], p[:], Alu.mult)
        nc.vector.tensor_scalar_add(t[:], p[:], 1.0)
        nc.vector.tensor_tensor(acc[:], acc[:], t[:], Alu.mult)
        a128 = constp.tile([HG, C], f32, tag=f"a128{par}")
        nc.vector.tensor_tensor(a128[:], p[:], p[:], Alu.mult)
        # d0_all = b0 * (1 + a0*geo)
        s0cf = constp.tile([HG, C], f32, tag=f"s0cf{par}")
        nc.vector.tensor_tensor(t[:], a0[:], acc[:], Alu.mult)
        nc.vector.tensor_scalar_add(s0cf[:], t[:], 1.0)
        d0_all = constp.tile([HG, C], f16, tag=f"d0a{par}")
        nc.vector.tensor_tensor(s0cf[:], s0cf[:], b0[:], Alu.mult)
        nc.vector.tensor_copy(d0_all[:], s0cf[:])
        # spacer = b0 * mask(a0>=T) * a0^129
        mask = constp.tile([HG, C], f32, tag=f"mask{par}")
        nc.vector.tensor_scalar(mask[:], a0[:], THRESH, None, Alu.is_ge)
        qcf = constp.tile([HG, C], f32, tag=f"qcf{par}")
        nc.vector.tensor_tensor(qcf[:], mask[:], a128[:], Alu.mult)
        nc.vector.tensor_tensor(qcf[:], qcf[:], a0[:], Alu.mult)
        nc.vector.tensor_tensor(qcf[:], qcf[:], b0[:], Alu.mult)
        # rec = 1/max(a0, T)
        rec = constp.tile([HG, C], f32, tag=f"rec{par}")
        nc.vector.tensor_scalar_max(t[:], a0[:], THRESH)
        nc.vector.reciprocal(rec[:], t[:])

        # g2u[w] for w=0..17, all channels: H-up matmul on the padded head
        # columns (two channel-halves to fit PSUM banks), then the W-blend
        # as one scalar_tensor_tensor per half.
        g2k18 = constp.tile([HG, C * 18], f32, tag=f"g2k18{par}")
        g2k18r = g2k18[:].rearrange("p (c w) -> p c w", c=C)
        for half in range(2):
            ch0 = half * (C // 2)
            psh = ps2p.tile([HG, (C // 2) * NG2C], f32, tag="ps2")
            nc.tensor.matmul(
                psh[:], u1[:],
                ghr[:, ch0:ch0 + C // 2], start=True, stop=True)

            c2s = constp.tile([HG, (C // 2) * NG2C], f32, tag=f"c2s{par}")
            nc.scalar.copy(c2s[:], psh[:])
            c2r = c2s[:].rearrange("p (c w) -> p c w", c=C // 2)
            # out[c, 2j+r] = 3*c2[j+1] + c2[j + 2r],  j=0..8  (3D APs only:
            # one scalar_tensor_tensor per W-parity)
            dst = g2k18r[:, ch0:ch0 + C // 2]
            nc.vector.scalar_tensor_tensor(
                dst[:, :, 0:17:2], c2r[:, :, 1:10], 3.0, c2r[:, :, 0:9],
                Alu.mult, Alu.add)
            nc.vector.scalar_tensor_tensor(
                dst[:, :, 1:18:2], c2r[:, :, 1:10], 3.0, c2r[:, :, 2:11],
                Alu.mult, Alu.add)

        # qd_all[c, 0] = 0; qd_all[c, 1..K] = g2u[c, w]*rec
        qd_all = constp.tile([HG, C * KP], f32, tag=f"qda{par}")
        nc.vector.memset(qd_all[:], 0.0)
        qdr = qd_all[:].rearrange("p (c w) -> p c w", c=C)
        nc.vector.tensor_tensor(
            qdr[:, :, 1:KP], g2k18r[:, :, 1:K + 1],
            rec[:].unsqueeze(-1).to_broadcast([HG, C, K]), Alu.mult)
        # qz_all: zeros except col0 = spacer
        qz_all = constp.tile([HG, C * KP], f16, tag=f"qza{par}")
        nc.vector.memset(qz_all[:], 0.0)
        qzr = qz_all[:].rearrange("p (c w) -> p c w", c=C)
        nc.vector.tensor_copy(qzr[:, :, 0:1], qcf[:].unsqueeze(-1))
        # all 32 correction scans in one go
        qo_all = constp.tile([HG, C * KP], f16, tag=f"qoa{par}")
        nc.vector.tensor_tensor_scan(
            qo_all[:], qd_all[:], qz_all[:], 0.0, Alu.mult, Alu.add)
        out[par] = (d0_all, qo_all)
    return out


def _emit(nc, pools, weights, dram):
    (ginp, ps1p, ps2p, xinp, g1sp, datp, outp, constp) = pools
    Xd, G12d, Auxd, Od = dram

    corr = _precompute(nc, ps2p, constp, weights, Auxd)

    for blk in range(NBLK):
        c0 = blk * B
        gb = ginp.tile([HG, B * 2 * WPAD], f16, tag="gb")
        gbr = gb[:].rearrange("p (c t w) -> p c t w", c=B, t=2)
        nc.scalar.dma_start(gbr, G12d[c0:c0 + B].transpose([1, 0, 2, 3]))

        xb2 = xinp.tile([HG, B * 2 * W], f16, tag="xb2")
        nc.scalar.dma_start(
            xb2[:].rearrange("p (c q w) -> p c q w", c=B, q=2),
            Xd[c0:c0 + B].rearrange("c (h q) w -> h c q w", q=2))

        for par in ("e", "o"):
            d0_all, qo_all = corr[par]
            u3 = weights[par + "3"]
            u1 = weights[par + "1"]
            pstart = 0 if par == "e" else 1

            # PE: H+W upsample straight into PSUM (g1u / g2u in separate
            # pools); one matmul pair per 2KB bank.
            ps1 = ps1p.tile([HG, B * W], f32, tag="ps1")
            ps2 = ps2p.tile([HG, B * W], f32, tag="ps2")
            for t, ps in ((0, ps1), (1, ps2)):
                for cp in range(B // 2):
                    dst = ps[:][:, cp * 2 * W:(cp * 2 + 2) * W]
                    center = _rep_ap(gbr[:, cp * 2, t, 1:2],
                                     [[2 * WPAD, 2], [1, HG], [0, 2]])
                    nc.tensor.matmul(dst, u3[:], center,
                                     start=True, stop=False)
                for cp in range(B // 2):
                    dst = ps[:][:, cp * 2 * W:(cp * 2 + 2) * W]
                    shift = _rep_ap(gbr[:, cp * 2, t, 0:1],
                                    [[2 * WPAD, 2], [1, HG], [2, 2]])
                    nc.tensor.matmul(dst, u1[:], shift,
                                     start=False, stop=True)
            g2u_r = ps2[:].rearrange("p (c w) -> p c w", c=B)
            # channel-seam reset for the scan carry (on ACT: the psum col0
            # holds finite matmul output, so mul-by-0 is a safe memset)
            nc.scalar.mul(g2u_r[:, :, 0:1], g2u_r[:, :, 0:1], 0.0)

            # ScalarE: g1u cast to fp16
            g1u = g1sp.tile([HG, B * W], f16, tag="g1u")
            nc.scalar.copy(g1u[:], ps1[:])

            g1ur = g1u[:].rearrange("p (c w) -> p c w", c=B)
            xb = xb2[:].rearrange("p (c q w) -> p c q w", c=B, q=2)[:, :, pstart]
            d = datp.tile([HG, B * W], f16, tag="d")
            dr = d[:].rearrange("p (c w) -> p c w", c=B)

            # d col0 (precomputed b0*s0c)
            nc.gpsimd.tensor_copy(
                dr[:, :, 0:1], d0_all[:, c0:c0 + B].unsqueeze(-1))
            # d = g1u*x in two pieces so the correction subtract (gpsimd)
            # overlaps the big tail multiply on the DVE
            nc.vector.tensor_tensor(
                dr[:, :, 1:K + 2], g1ur[:, :, 1:K + 2], xb[:, :, 1:K + 2],
                Alu.mult)
            qor = qo_all[:].rearrange("p (c w) -> p c w", c=C)
            nc.vector.tensor_tensor(
                dr[:, :, 1:KP], dr[:, :, 1:KP],
                qor[:, c0:c0 + B, 1:KP], Alu.subtract)
            nc.vector.tensor_tensor(
                dr[:, :, K + 2:], g1ur[:, :, K + 2:], xb[:, :, K + 2:],
                Alu.mult)

            # main scan: s[x] = g2u[x]*s[x-1] + d[x]
            ot = outp.tile([HG, B * W], f16, tag="ot")
            nc.vector.tensor_tensor_scan(
                ot[:], ps2[:], d[:], 0.0, Alu.mult, Alu.add)
            nc.sync.dma_start(
                Od[c0:c0 + B, pstart:H:2, :].transpose([1, 0, 2]),
                ot[:].rearrange("p (c w) -> p c w", c=B))


def build():
    nc = bacc.Bacc("TRN2", target_bir_lowering=False, debug=False,
                   num_devices=NCORES)
    Xd = nc.dram_tensor("X", [C, H, W], f16, kind="ExternalInput")
    G12d = nc.dram_tensor("G12", [C, HG, 2, WPAD], f16, kind="ExternalInput")
    Auxd = nc.dram_tensor("AUX", [HG, C * (2 + NG2C + 2)], f16,
                          kind="ExternalInput")
    Ud = {n: nc.dram_tensor(n.upper(), [HG, HG], f16, kind="ExternalInput")
          for n in ("e3", "e1", "o3", "o1")}
    Od = nc.dram_tensor("O", [C, H, W], f16, kind="ExternalOutput")

    with tile.TileContext(nc) as tc:
        with (
            tc.tile_pool(name="const", bufs=1) as constp,
            tc.tile_pool(name="gin", bufs=5) as ginp,
            tc.tile_pool(name="ps1", bufs=2, space="PSUM") as ps1p,
            tc.tile_pool(name="ps2", bufs=2, space="PSUM") as ps2p,
            tc.tile_pool(name="xin", bufs=4) as xinp,
            tc.tile_pool(name="g1s", bufs=4) as g1sp,
            tc.tile_pool(name="dat", bufs=4) as datp,
            tc.tile_pool(name="out", bufs=4) as outp,
        ):
            weights = {}
            for n in ("e3", "e1", "o3", "o1"):
                w = constp.tile([HG, HG], f16, tag=f"u{n}")
                nc.sync.dma_start(w[:], Ud[n][:])
                weights[n] = w
            pools = (ginp, ps1p, ps2p, xinp, g1sp, datp, outp, constp)
            _emit(nc, pools, weights, (Xd, G12d, Auxd, Od))

    nc.compile()
    return nc


_NC = None


def kernel(X, G1, G2, G3=None, **_):
    global _NC
    if _NC is None:
        _NC = build()
    ue, uo = _upsample_mats()
    wmats = {"E3": (3.0 * ue).astype(np.float16),
             "E1": ue.astype(np.float16),
             "O3": (3.0 * uo).astype(np.float16),
             "O1": uo.astype(np.float16)}

    def pad(G):
        return np.concatenate([G[..., :1], G, G[..., -1:]], axis=-1)

    Xh = np.ascontiguousarray(X).astype(np.float16)
    G12h = np.stack([pad(np.asarray(G1)), pad(np.asarray(G2))],
                    axis=3).astype(np.float16)
    # host-packed aux: per (h-row, channel): G1 col0, G2 col0, padded G2
    # head cols 0..NG2C-1, X col0 (even rows), X col0 (odd rows)
    FA = 2 + NG2C + 2
    aux = np.empty((NCORES, HG, C, FA), np.float16)
    aux[..., 0] = G12h[:, :, :, 0, 1].transpose(0, 2, 1)
    aux[..., 1] = G12h[:, :, :, 1, 1].transpose(0, 2, 1)
    aux[..., 2:2 + NG2C] = G12h[:, :, :, 1, 0:NG2C].transpose(0, 2, 1, 3)
    aux[..., FA - 2] = Xh[:, :, 0::2, 0].transpose(0, 2, 1)
    aux[..., FA - 1] = Xh[:, :, 1::2, 0].transpose(0, 2, 1)
    aux = aux.reshape(NCORES, HG, C * FA)

    in_maps = [
        {"X": Xh[k], "G12": np.ascontiguousarray(G12h[k]),
         "AUX": np.ascontiguousarray(aux[k]), **wmats}
        for k in range(NCORES)
    ]
    res = run_bass_kernel_spmd(_NC, in_maps, list(range(NCORES)))
    kernel.last_result = res
    out = np.stack([res.results[k]["O"] for k in range(NCORES)])
    return out.astype(np.float32)


# revision 25
# speedup vs baseline: 1.2273x; 1.2273x over previous
"""GateRecurrent2dnoind (horizontal, forward) Trainium2 kernel, v6.

Semantics (matching the reference):
  G1u, G2u = bilinear 2x upsample (half-pixel) of G1, G2 to (256, 256)
  g1x = G1u * X
  o = g1x; repeat 128x: o = g1x + G2u * shift_right_w(o)   (left edge replicated)

The 128 Jacobi passes collapse into ONE sequential scan along W with an exact
depth-128 window emulation:
  s[x] = d[x] + g2u[x]*s[x-1],  d = g1u*X  with
  d[0] scaled by s0c = 1 + a0*sum_{m=0}^{127} a0^m   (a0 = g2u[.,0]) and
  d[x] -= q[x] for x=1..K, q[x] = (b0*qc)*prod_{i=1..x}(g2u[i]*rec),
  qc = mask(a0>=T)*a0^129, rec = 1/max(a0,T)  (window-excess correction).

Design highlights (measured on HW):
  - fp16 end-to-end for the big tensors (rel err 1.5e-3 vs the 2e-2 gate).
  - Both upsample directions run on the TensorEngine: per 2KB PSUM bank,
    matmul#1 (weights 3U, moving AP with a stride-0 repeat dim) writes the
    center tap to both W-parity slots, matmul#2 (weights U, a (j:+1, rep:+2)
    window over host-padded 130-col G tiles) accumulates the side taps.
  - The main scan reads g2u straight from PSUM (fp32 data0 = 2 cyc/elem;
    16-bit data0 would be 4 cyc/elem), data1 = d fp16, out fp16.
  - The ENTIRE correction chain (b0, spacers, qd, and all 32 correction
    scans) is precomputed per parity from G1/G2/X column slices, so the
    steady-state loop is only: 8 matmuls (PE), g1u cast (ACT), memset +
    2-piece d-multiply + main scan (DVE), col0 copy + subtract (GpSimd).

Sharding: batch b -> core b (8 batches, 8 cores). Per core: [64, 256, 256].
"""

import numpy as np

import concourse.bacc as bacc
import concourse.mybir as mybir
import concourse.tile as tile
from concourse.ap import AP
from concourse.bass_utils import run_bass_kernel_spmd

f32 = mybir.dt.float32
f16 = mybir.dt.float16
Alu = mybir.AluOpType

NCORES = 8
C = 64          # channels per core
H = 256
W = 256
HG = 128        # G input h/w
WPAD = HG + 2   # G w + replicate pads
B = 4           # channels per block
NBLK = C // B
K = 16          # correction columns
KP = K + 1      # correction scan width per channel
THRESH = 0.75   # a0 mask/clamp for the correction chain
NG2C = 11       # padded G2 columns needed for g2u[0..17] (pad + cols 0..9)


def _upsample_mats():
    """[k=in_row, m=out_row] H-upsample matrices, scaled by 0.25.

    even rows: out[m] = 0.25*in[m-1] + 0.75*in[m]   (m=0 clamps to in[0])
    odd rows:  out[m] = 0.75*in[m] + 0.25*in[m+1]   (m=127 clamps to in[127])
    """
    ue = np.zeros((HG, HG), np.float32)
    uo = np.zeros((HG, HG), np.float32)
    for m in range(HG):
        ue[m, m] += 0.25 * 0.75
        ue[max(m - 1, 0), m] += 0.25 * 0.25
        uo[m, m] += 0.25 * 0.75
        uo[min(m + 1, HG - 1), m] += 0.25 * 0.25
    return ue, uo


def _rep_ap(anchor, dims):
    """Raw AP sharing anchor's tensor/offset/partition dim, custom free dims."""
    return AP(anchor.tensor, anchor.offset, [list(anchor.ap[0])] + dims)


def _precompute(nc, ps2p, constp, weights, Auxd):
    """Boundary coefficients + full correction tables, both parities merged
    into single wide ops (layout [(par)(c)] on the free dim) to shorten the
    serial DVE chain at startup.

    Aux layout (host-packed, [128, FA per channel] fp16 c-major):
      [c,0] = G1[c,:,0]; [c,1] = G2[c,:,0]; [c,2:2+NG2C] = padded G2 head;
      [c,-2] = X[c,0::2,0]; [c,-1] = X[c,1::2,0].
    Returns (d0_all [128, 2C] f16, qo_all [128, 2C*KP] f16).
    """
    FA = 2 + NG2C + 2
    C2 = 2 * C
    aux = constp.tile([HG, C * FA], f16, tag="aux")
    nc.sync.dma_start(aux[:], Auxd[:])
    auxr = aux[:].rearrange("p (c f) -> p c f", c=C)

    # a0 / g1u0 for both parities via paired matmuls into one PSUM bank
    psa = ps2p.tile([HG, C2], f32, tag="ps2")
    psb = ps2p.tile([HG, C2], f32, tag="ps2")
    for pi, par in enumerate(("e", "o")):
        u1 = weights[par + "1"]
        st = pi == 0
        nc.tensor.matmul(psa[:][:, pi * C:(pi + 1) * C], u1[:],
                         auxr[:, :, 1], start=st, stop=not st)
        nc.tensor.matmul(psb[:][:, pi * C:(pi + 1) * C], u1[:],
                         auxr[:, :, 0], start=st, stop=not st)
    a0 = constp.tile([HG, C2], f32, tag="a0")
    nc.vector.tensor_scalar_mul(a0[:], psa[:], 4.0)
    b0 = constp.tile([HG, C2], f32, tag="b0")
    nc.vector.tensor_scalar_mul(b0[:], psb[:], 4.0)
    xc0f = constp.tile([HG, C2], f32, tag="xc0f")
    nc.vector.tensor_copy(xc0f[:][:, 0:C], auxr[:, :, FA - 2])
    nc.vector.tensor_copy(xc0f[:][:, C:C2], auxr[:, :, FA - 1])
    nc.vector.tensor_tensor(b0[:], b0[:], xc0f[:], Alu.mult)

    # geo = sum_{m=0}^{127} a0^m = prod_k (1 + a0^(2^k)), k=0..6
    acc = constp.tile([HG, C2], f32, tag="acc")
    p = constp.tile([HG, C2], f32, tag="p")
    t = constp.tile([HG, C2], f32, tag="t")
    nc.vector.tensor_scalar_add(acc[:], a0[:], 1.0)
    nc.vector.tensor_tensor(p[:], a0[:], a0[:], Alu.mult)
    for _ in range(5):
        nc.vector.tensor_scalar_add(t[:], p[:], 1.0)
        nc.vector.tensor_tensor(acc[:], acc[:], t[:], Alu.mult)
        nc.vector.tensor_tensor(p[:], p[:], p[:], Alu.mult)
    nc.vector.tensor_scalar_add(t[:], p[:], 1.0)
    nc.vector.tensor_tensor(acc[:], acc[:], t[:], Alu.mult)
    a128 = constp.tile([HG, C2], f32, tag="a128")
    nc.vector.tensor_tensor(a128[:], p[:], p[:], Alu.mult)
    # d0_all = b0 * (1 + a0*geo)
    s0cf = constp.tile([HG, C2], f32, tag="s0cf")
    nc.vector.tensor_tensor(t[:], a0[:], acc[:], Alu.mult)
    nc.vector.tensor_scalar_add(s0cf[:], t[:], 1.0)
    d0_all = constp.tile([HG, C2], f16, tag="d0a")
    nc.vector.tensor_tensor(s0cf[:], s0cf[:], b0[:], Alu.mult)
    nc.vector.tensor_copy(d0_all[:], s0cf[:])
    # spacer = b0 * mask(a0>=T) * a0^129
    mask = constp.tile([HG, C2], f32, tag="mask")
    nc.vector.tensor_scalar(mask[:], a0[:], THRESH, None, Alu.is_ge)
    qcf = constp.tile([HG, C2], f32, tag="qcf")
    nc.vector.tensor_tensor(qcf[:], mask[:], a128[:], Alu.mult)
    nc.vector.tensor_tensor(qcf[:], qcf[:], a0[:], Alu.mult)
    nc.vector.tensor_tensor(qcf[:], qcf[:], b0[:], Alu.mult)
    # rec = 1/max(a0, T)
    rec = constp.tile([HG, C2], f32, tag="rec")
    nc.vector.tensor_scalar_max(t[:], a0[:], THRESH)
    nc.vector.reciprocal(rec[:], t[:])

    # g2u[w] for w=0..17, all channels/parities: H-up matmul on the padded
    # head columns (channel-halves fit PSUM banks), W-blend as two stt per
    # half (3D APs only).
    g2k18 = constp.tile([HG, C2 * 18], f32, tag="g2k18")
    g2k18r = g2k18[:].rearrange("p (c w) -> p c w", c=C2)
    for pi, par in enumerate(("e", "o")):
        u1 = weights[par + "1"]
        for half in range(2):
            ch0 = half * (C // 2)
            psh = ps2p.tile([HG, (C // 2) * NG2C], f32, tag="ps2")
            nc.tensor.matmul(
                psh[:], u1[:],
                auxr[:, ch0:ch0 + C // 2, 2:2 + NG2C], start=True, stop=True)
            c2s = constp.tile([HG, (C // 2) * NG2C], f32, tag="c2s")
            nc.scalar.copy(c2s[:], psh[:])
            c2r = c2s[:].rearrange("p (c w) -> p c w", c=C // 2)
            dst = g2k18r[:, pi * C + ch0:pi * C + ch0 + C // 2]
            nc.vector.scalar_tensor_tensor(
                dst[:, :, 0:17:2], c2r[:, :, 1:10], 3.0, c2r[:, :, 0:9],
                Alu.mult, Alu.add)
            nc.vector.scalar_tensor_tensor(
                dst[:, :, 1:18:2], c2r[:, :, 1:10], 3.0, c2r[:, :, 2:11],
                Alu.mult, Alu.add)

    # qd_all[pc, 0] = 0; qd_all[pc, 1..K] = g2u[pc, w]*rec
    qd_all = constp.tile([HG, C2 * KP], f32, tag="qda")
    nc.vector.memset(qd_all[:], 0.0)
    qdr = qd_all[:].rearrange("p (c w) -> p c w", c=C2)
    nc.vector.tensor_tensor(
        qdr[:, :, 1:KP], g2k18r[:, :, 1:K + 1],
        rec[:].unsqueeze(-1).to_broadcast([HG, C2, K]), Alu.mult)
    # qz_all: zeros except col0 = spacer
    qz_all = constp.tile([HG, C2 * KP], f16, tag="qza")
    nc.vector.memset(qz_all[:], 0.0)
    qzr = qz_all[:].rearrange("p (c w) -> p c w", c=C2)
    nc.vector.tensor_copy(qzr[:, :, 0:1], qcf[:].unsqueeze(-1))
    # all correction scans (both parities, 32 blocks) in one op
    qo_all = constp.tile([HG, C2 * KP], f16, tag="qoa")
    nc.vector.tensor_tensor_scan(
        qo_all[:], qd_all[:], qz_all[:], 0.0, Alu.mult, Alu.add)
    return d0_all, qo_all


def _emit(nc, pools, weights, dram):
    (ginp, ps1p, ps2p, xinp, g1sp, datp, outp, constp) = pools
    Xd, G12d, Auxd, Od = dram

    d0_all, qo_all = _precompute(nc, ps2p, constp, weights, Auxd)

    for blk in range(NBLK):
        c0 = blk * B
        gb = ginp.tile([HG, B * 2 * WPAD], f16, tag="gb")
        gbr = gb[:].rearrange("p (c t w) -> p c t w", c=B, t=2)
        nc.scalar.dma_start(gbr, G12d[c0:c0 + B].transpose([1, 0, 2, 3]))

        xb2 = xinp.tile([HG, B * 2 * W], f16, tag="xb2")
        nc.scalar.dma_start(
            xb2[:].rearrange("p (c q w) -> p c q w", c=B, q=2),
            Xd[c0:c0 + B].rearrange("c (h q) w -> h c q w", q=2))

        for pi, par in enumerate(("e", "o")):
            u3 = weights[par + "3"]
            u1 = weights[par + "1"]
            pstart = 0 if par == "e" else 1

            # PE: H+W upsample straight into PSUM (g1u / g2u in separate
            # pools); one matmul pair per 2KB bank.
            ps1 = ps1p.tile([HG, B * W], f32, tag="ps1")
            ps2 = ps2p.tile([HG, B * W], f32, tag="ps2")
            for t, ps in ((0, ps1), (1, ps2)):
                for cp in range(B // 2):
                    dst = ps[:][:, cp * 2 * W:(cp * 2 + 2) * W]
                    center = _rep_ap(gbr[:, cp * 2, t, 1:2],
                                     [[2 * WPAD, 2], [1, HG], [0, 2]])
                    nc.tensor.matmul(dst, u3[:], center,
                                     start=True, stop=False)
                for cp in range(B // 2):
                    dst = ps[:][:, cp * 2 * W:(cp * 2 + 2) * W]
                    shift = _rep_ap(gbr[:, cp * 2, t, 0:1],
                                    [[2 * WPAD, 2], [1, HG], [2, 2]])
                    nc.tensor.matmul(dst, u1[:], shift,
                                     start=False, stop=True)
            g2u_r = ps2[:].rearrange("p (c w) -> p c w", c=B)
            # channel-seam reset for the scan carry (on ACT: the psum col0
            # holds finite matmul output, so mul-by-0 is a safe memset)
            nc.scalar.mul(g2u_r[:, :, 0:1], g2u_r[:, :, 0:1], 0.0)

            # ScalarE: g1u cast to fp16
            g1u = g1sp.tile([HG, B * W], f16, tag="g1u")
            nc.scalar.copy(g1u[:], ps1[:])

            g1ur = g1u[:].rearrange("p (c w) -> p c w", c=B)
            xb = xb2[:].rearrange("p (c q w) -> p c q w", c=B, q=2)[:, :, pstart]
            d = datp.tile([HG, B * W], f16, tag="d")
            dr = d[:].rearrange("p (c w) -> p c w", c=B)

            # d col0 (precomputed b0*s0c)
            nc.gpsimd.tensor_copy(
                dr[:, :, 0:1],
                d0_all[:, pi * C + c0:pi * C + c0 + B].unsqueeze(-1))
            # d = g1u*x in two pieces so the correction subtract (gpsimd)
            # overlaps the big tail multiply on the DVE
            nc.vector.tensor_tensor(
                dr[:, :, 1:K + 2], g1ur[:, :, 1:K + 2], xb[:, :, 1:K + 2],
                Alu.mult)
            qor = qo_all[:].rearrange("p (c w) -> p c w", c=2 * C)
            nc.gpsimd.tensor_tensor(
                dr[:, :, 1:KP], dr[:, :, 1:KP],
                qor[:, pi * C + c0:pi * C + c0 + B, 1:KP], Alu.subtract)
            nc.vector.tensor_tensor(
                dr[:, :, K + 2:], g1ur[:, :, K + 2:], xb[:, :, K + 2:],
                Alu.mult)

            # main scan: s[x] = g2u[x]*s[x-1] + d[x]
            ot = outp.tile([HG, B * W], f16, tag="ot")
            nc.vector.tensor_tensor_scan(
                ot[:], ps2[:], d[:], 0.0, Alu.mult, Alu.add)
            nc.sync.dma_start(
                Od[c0:c0 + B, pstart:H:2, :].transpose([1, 0, 2]),
                ot[:].rearrange("p (c w) -> p c w", c=B))


def build():
    nc = bacc.Bacc("TRN2", target_bir_lowering=False, debug=False,
                   num_devices=NCORES)
    Xd = nc.dram_tensor("X", [C, H, W], f16, kind="ExternalInput")
    G12d = nc.dram_tensor("G12", [C, HG, 2, WPAD], f16, kind="ExternalInput")
    Auxd = nc.dram_tensor("AUX", [HG, C * (2 + NG2C + 2)], f16,
                          kind="ExternalInput")
    Ud = {n: nc.dram_tensor(n.upper(), [HG, HG], f16, kind="ExternalInput")
          for n in ("e3", "e1", "o3", "o1")}
    Od = nc.dram_tensor("O", [C, H, W], f16, kind="ExternalOutput")

    with tile.TileContext(nc) as tc:
        with (
            tc.tile_pool(name="const", bufs=1) as constp,
            tc.tile_pool(name="gin", bufs=5) as ginp,
            tc.tile_pool(name="ps1", bufs=2, space="PSUM") as ps1p,
            tc.tile_pool(name="ps2", bufs=2, space="PSUM") as ps2p,
            tc.tile_pool(name="xin", bufs=4) as xinp,
            tc.tile_pool(name="g1s", bufs=4) as g1sp,
            tc.tile_pool(name="dat", bufs=4) as datp,
            tc.tile_pool(name="out", bufs=4) as outp,
        ):
            weights = {}
            for n in ("e3", "e1", "o3", "o1"):
                w = constp.tile([HG, HG], f16, tag=f"u{n}")
                nc.sync.dma_start(w[:], Ud[n][:])
                weights[n] = w
            pools = (ginp, ps1p, ps2p, xinp, g1sp, datp, outp, constp)
            _emit(nc, pools, weights, (Xd, G12d, Auxd, Od))

    nc.compile()
    return nc


_NC = None


def kernel(X, G1, G2, G3=None, **_):
    global _NC
    if _NC is None:
        _NC = build()
    ue, uo = _upsample_mats()
    wmats = {"E3": (3.0 * ue).astype(np.float16),
             "E1": ue.astype(np.float16),
             "O3": (3.0 * uo).astype(np.float16),
             "O1": uo.astype(np.float16)}

    def pad(G):
        return np.concatenate([G[..., :1], G, G[..., -1:]], axis=-1)

    Xh = np.ascontiguousarray(X).astype(np.float16)
    G12h = np.stack([pad(np.asarray(G1)), pad(np.asarray(G2))],
                    axis=3).astype(np.float16)
    # host-packed aux: per (h-row, channel): G1 col0, G2 col0, padded G2
    # head cols 0..NG2C-1, X col0 (even rows), X col0 (odd rows)
    FA = 2 + NG2C + 2
    aux = np.empty((NCORES, HG, C, FA), np.float16)
    aux[..., 0] = G12h[:, :, :, 0, 1].transpose(0, 2, 1)
    aux[..., 1] = G12h[:, :, :, 1, 1].transpose(0, 2, 1)
    aux[..., 2:2 + NG2C] = G12h[:, :, :, 1, 0:NG2C].transpose(0, 2, 1, 3)
    aux[..., FA - 2] = Xh[:, :, 0::2, 0].transpose(0, 2, 1)
    aux[..., FA - 1] = Xh[:, :, 1::2, 0].transpose(0, 2, 1)
    aux = aux.reshape(NCORES, HG, C * FA)

    in_maps = [
        {"X": Xh[k], "G12": np.ascontiguousarray(G12h[k]),
         "AUX": np.ascontiguousarray(aux[k]), **wmats}
        for k in range(NCORES)
    ]
    res = run_bass_kernel_spmd(_NC, in_maps, list(range(NCORES)))
    kernel.last_result = res
    out = np.stack([res.results[k]["O"] for k in range(NCORES)])
    return out.astype(np.float32)
